# revision 57
# baseline (speedup 1.0000x reference)
"""Trainium2 Bass kernel for additive (Bahdanau-style) attention.

Reference computation (per batch b, over T time steps):
    h[b,t,:]  = values[b,t,:] @ W1_k + W1_b + (query[b,:] @ W2_k + W2_b)   # [U]
    score     = tanh(h) @ V_k + V_b                                        # [B,T,1]
    attn      = softmax(score, axis=1)
    context   = attn[:, T-1] * values[:, T-1]                              # [B,D]
    returns (context, attn)

Sharding: data-parallel over batch B=32 across 8 cores (B_loc=4/core),
weights replicated.  No collectives.

Kernel layout (per core): h is computed TRANSPOSED (h^T: U on partitions,
tokens on the free axis) so that
  - W1 is the matmul stationary operand in its natural [D, U] layout,
  - the U-reduction for the score is a PE matmul with lhsT = V[128,1],
  - softmax over T is a free-axis operation.
All inputs are loaded through HWDGE as f32 and cast to bf16 on the vector
engine (SWDGE descriptor generation on the Q7s serializes ~1us per DMA and
HWDGE transfers complete in issue order, so the emission order below is the
HBM arrival order).  values tiles are transposed on-chip via PE
transpose-mode; all matmuls run in bf16 with fp32 PSUM accumulation; the
softmax runs in fp32.  V_b is dropped: softmax is shift-invariant and score
itself is not an output.
"""

import sys
import types

import numpy as np

import concourse.bass as bass
import concourse.tile as tile
from concourse import bacc, mybir
from concourse import masks


def _ensure_ntff_hook_module():
    """bass_utils unconditionally imports antenv.axon_hooks when tracing is
    requested (e.g. BASS_TRACE=1); the image's antenv lacks it.  Register a
    compatible module so tracing works when the axon .so supports it and
    degrades to a logged no-op otherwise."""
    try:
        import antenv.axon_hooks  # noqa: F401
        return
    except ImportError:
        pass
    mod = types.ModuleType("antenv.axon_hooks")
    state = {"hook": None}
    mod.set_axon_ntff_profile_hook = lambda h: state.__setitem__("hook", h)
    mod.get_axon_ntff_profile_hook = lambda: state["hook"]
    sys.modules["antenv.axon_hooks"] = mod
    try:
        import antenv
        antenv.axon_hooks = mod
        from trn_agent_boot.trn_boot import _ntff_profile_via_ctypes
        mod.set_axon_ntff_profile_hook(
            _ntff_profile_via_ctypes("/opt/axon/libaxon_pjrt.so"))
    except Exception:
        pass  # hook stays None; bass_utils logs and skips tracing


_ensure_ntff_hook_module()
from concourse.bass_utils import run_bass_kernel_spmd  # noqa: E402

F32 = mybir.dt.float32
BF16 = mybir.dt.bfloat16
AFT = mybir.ActivationFunctionType

B, T, D, U = 32, 2048, 1024, 1024
N_CORES = 8
B_LOC = B // N_CORES          # 4 batches per core
TOKS = B_LOC * T              # 8192 tokens per core
TOK_TILE = 512                # tokens per compute tile
N_TS = TOKS // TOK_TILE       # 16 token tiles per core
TS_PER_B = T // TOK_TILE      # 4 token tiles per batch
KC = D // 128                 # 8 contraction chunks
UC = U // 128                 # 8 U chunks
NPREF = 3                     # token tiles prefetched before the main loop


def build_nc():
    nc = bacc.Bacc("TRN2", target_bir_lowering=False, debug=False,
                   num_devices=N_CORES)

    values = nc.dram_tensor("values", [B_LOC, T, D], F32, kind="ExternalInput")
    query = nc.dram_tensor("query", [B_LOC, D], F32, kind="ExternalInput")
    w1 = nc.dram_tensor("W1_k", [D, U], F32, kind="ExternalInput")
    w1b = nc.dram_tensor("W1_b", [U], F32, kind="ExternalInput")
    w2 = nc.dram_tensor("W2_k", [D, U], F32, kind="ExternalInput")
    w2b = nc.dram_tensor("W2_b", [U], F32, kind="ExternalInput")
    vk = nc.dram_tensor("V_k", [U, 1], F32, kind="ExternalInput")
    out_attn = nc.dram_tensor("out_attn", [B_LOC, T], F32, kind="ExternalOutput")
    out_ctx = nc.dram_tensor("out_ctx", [B_LOC, D], F32, kind="ExternalOutput")

    vals_flat = values.ap().rearrange("b t d -> (b t) d")

    from contextlib import ExitStack
    with tile.TileContext(nc) as tc:
        with ExitStack() as stk:
            ep = lambda **kw: stk.enter_context(tc.tile_pool(**kw))
            const_pool = ep(name="const", bufs=1)
            w1_pool = ep(name="w1p", bufs=1)
            w2_pool = ep(name="w2p", bufs=1)
            pre_pool = ep(name="pre", bufs=1)
            vstage_pool = ep(name="vstage", bufs=2)
            wstage_pool = ep(name="wstage", bufs=4)
            vnat_pool = ep(name="vnat", bufs=3)
            vt_pool = ep(name="vt", bufs=2 * KC)
            tanh_pool = ep(name="tanh", bufs=9)
            exp_pool = ep(name="expb", bufs=2)
            attn_pool = ep(name="attn", bufs=2)
            ctx_pool = ep(name="ctx", bufs=1)
            vlast_pool = ep(name="vlast", bufs=1)
            small_pool = ep(name="small", bufs=8)
            scpy_pool = ep(name="scpy", bufs=2)
            psum_t = ep(name="ps_t", bufs=3, space="PSUM")
            psum_h = ep(name="ps_h", bufs=3, space="PSUM")
            psum_s = ep(name="ps_s", bufs=2, space="PSUM")
            def cast_load(dst_slice, src_f32_ap, pool, tag, shape):
                stg = pool.tile(shape, F32, tag=tag)
                parts = src_f32_ap.shape[0]
                free = src_f32_ap.shape[-1]
                nc.sync.dma_start(stg[:parts, :free], src_f32_ap)
                nc.vector.tensor_copy(dst_slice, stg[:parts, :free])

            def load_vnat(tok0):
                """512-token values tile as ONE [128, 4*D] load (partition p
                holds tokens tok0+p, +128, +256, +384): single HWDGE DMA +
                single DVE bf16 cast amortize the ~0.6us per-DMA setup."""
                vn = vnat_pool.tile([128, 4 * D], BF16, tag="vnat")
                stg = vstage_pool.tile([128, 4 * D], F32, tag="vstage")
                nc.sync.dma_start(
                    stg[:].rearrange("p (j d) -> p j d", j=4),
                    vals_flat[tok0:tok0 + TOK_TILE, :].rearrange(
                        "(j p) d -> p j d", p=128))
                nc.vector.tensor_copy(vn[:], stg[:])
                return vn

            def emit_transposes(vn):
                """merged [128, 4*D] tile -> 8 [d-chunk, 512-token] tiles."""
                vt = []
                for k in range(KC):
                    pt = psum_t.tile([128, TOK_TILE], BF16, tag="ps_t")
                    for j in range(4):
                        nc.tensor.transpose(
                            pt[:, j * 128:(j + 1) * 128],
                            vn[:, j * D + k * 128: j * D + (k + 1) * 128],
                            ident[:])
                    v = vt_pool.tile([128, TOK_TILE], BF16, tag="vt")
                    nc.vector.tensor_copy(v[:], pt[:])
                    vt.append(v)
                return vt

            def emit_mains(vt, u):
                ph = psum_h.tile([128, TOK_TILE], F32, tag="ps_h")
                for k in range(KC):
                    nc.tensor.matmul(
                        ph[:],
                        w1sb[:, k * U + u * 128: k * U + (u + 1) * 128],
                        vt[k][:],
                        start=(k == 0), stop=(k == KC - 1))
                return ph

            def emit_tanh(ph, u, b):
                th = tanh_pool.tile([128, TOK_TILE], BF16, tag="th")
                nc.scalar.activation(
                    th[:], ph[:], AFT.Tanh,
                    bias=hbias[:, u * B_LOC + b: u * B_LOC + b + 1])
                return th

            # ---- preamble; emission order == HBM arrival order ----
            ident = const_pool.tile([128, 128], BF16)
            masks.make_identity(nc, ident[:])

            # e4[p] = 1 at p in {0,32,64,96}: reduces the 4 packed score rows
            e4a = const_pool.tile([128, 1], BF16)
            nc.vector.tensor_add(e4a[:], ident[:, 0:1], ident[:, 32:33])
            e4b = const_pool.tile([128, 1], BF16)
            nc.vector.tensor_add(e4b[:], ident[:, 64:65], ident[:, 96:97])
            e4 = const_pool.tile([128, 1], BF16)
            nc.vector.tensor_add(e4[:], e4a[:], e4b[:])

            qsb = const_pool.tile([B_LOC, D], BF16)
            cast_load(qsb[:], query.ap()[:, :], wstage_pool, "qstage",
                      [B_LOC, D])

            # biases as CONTIGUOUS [1,1024] rows (the "(u p) -> p u"
            # strided load costs 1024 4-byte descriptors); their add into h
            # rides the q2 matmul as a K=1 ones-row term.
            b1 = pre_pool.tile([1, U], F32)
            nc.sync.dma_start(b1[:], w1b.ap().rearrange("(o u) -> o u", o=1))
            b2 = pre_pool.tile([1, U], F32)
            nc.sync.dma_start(b2[:], w2b.ap().rearrange("(o u) -> o u", o=1))
            brow = pre_pool.tile([1, U], BF16)
            nc.vector.tensor_add(brow[:], b1[:], b2[:])
            ones1 = pre_pool.tile([1, B_LOC], BF16)
            nc.vector.memset(ones1[:], 1.0)

            vsb = const_pool.tile([128, UC], BF16)
            vkrow = pre_pool.tile([1, U], BF16)
            cast_load(vkrow[:], vk.ap().rearrange("u o -> o u"),
                      wstage_pool, "vkstage", [1, U])

            # values ts0, then W1 — everything the first main matmuls need
            vnat_pre = [load_vnat(0)]
            w1sb = w1_pool.tile([128, KC * U], BF16)
            for kg in range(2):
                stg = vstage_pool.tile([128, 4 * D], F32, tag="vstage")
                nc.sync.dma_start(
                    stg[:].rearrange("p (k u) -> p k u", k=4),
                    w1.ap()[kg * 512:(kg + 1) * 512, :].rearrange(
                        "(k p) u -> p k u", p=128))
                for k in range(4):
                    nc.vector.tensor_copy(
                        w1sb[:, (kg * 4 + k) * U: (kg * 4 + k + 1) * U],
                        stg[:, k * U:(k + 1) * U])

            # query^T (tiny PE work, can run as soon as qsb+ident land)
            qt = pre_pool.tile([128, KC * B_LOC], BF16)
            for k in range(KC):
                pq = psum_t.tile([128, B_LOC], BF16, tag="ps_t")
                nc.tensor.transpose(pq[:], qsb[:, k * 128:(k + 1) * 128],
                                    ident[0:B_LOC, 0:B_LOC])
                nc.vector.tensor_copy(qt[:, k * B_LOC:(k + 1) * B_LOC], pq[:])

            for u in range(UC):
                pv = psum_t.tile([128, 1], BF16, tag="ps_t")
                nc.tensor.transpose(pv[:], vkrow[0:1, u * 128:(u + 1) * 128],
                                    ident[0:1, 0:1])
                nc.vector.tensor_copy(vsb[:, u:u + 1], pv[:])

            # ts0: transposes + first three u-chunks of h^T, emitted BEFORE
            # the W2/q2 block so the PE starts as soon as W1+values land.
            hbias = const_pool.tile([128, UC * B_LOC], F32)
            vt0 = emit_transposes(vnat_pre[0])
            phs0 = [emit_mains(vt0, u) for u in range(3)]

            # W2 by column halves: hbias[u<4] only needs W2[:, 0:512], so the
            # first tanh unblocks after half the W2 traffic.
            # hbias[p, u*B_LOC+b] = q2[b, u*128+p] + W1_b[...] + W2_b[...]
            w2sb = w2_pool.tile([128, KC * U], BF16)
            q2sb = pre_pool.tile([B_LOC, U], BF16)
            for n in range(2):
                stg2 = vstage_pool.tile([128, 4 * D], F32, tag="vstage")
                nc.sync.dma_start(
                    stg2[:, 0:KC * 512].rearrange("p (k u) -> p k u", k=KC),
                    w2.ap()[:, n * 512:(n + 1) * 512].rearrange(
                        "(k p) u -> p k u", p=128))
                for k in range(KC):
                    nc.vector.tensor_copy(
                        w2sb[:, k * U + n * 512: k * U + (n + 1) * 512],
                        stg2[:, k * 512:(k + 1) * 512])
                pq2 = psum_s.tile([B_LOC, 512], F32, tag="ps_s")
                for k in range(KC):
                    nc.tensor.matmul(
                        pq2[:],
                        qt[:, k * B_LOC:(k + 1) * B_LOC],
                        w2sb[:, k * U + n * 512: k * U + (n + 1) * 512],
                        start=(k == 0), stop=False)
                nc.tensor.matmul(pq2[:], ones1[:],
                                 brow[0:1, n * 512:(n + 1) * 512],
                                 start=False, stop=True)
                nc.vector.tensor_copy(q2sb[:, n * 512:(n + 1) * 512], pq2[:])
                for u in range(n * 4, n * 4 + 4):
                    pq2t = psum_t.tile([128, B_LOC], BF16, tag="ps_t")
                    nc.tensor.transpose(pq2t[:], q2sb[:, u * 128:(u + 1) * 128],
                                        ident[0:B_LOC, 0:B_LOC])
                    nc.scalar.activation(
                        hbias[:, u * B_LOC:(u + 1) * B_LOC], pq2t[:],
                        AFT.Identity)
                if n == 0:
                    vnat_pre.append(load_vnat(TOK_TILE))

            for ts in range(2, NPREF):
                vnat_pre.append(load_vnat(ts * TOK_TILE))

            # ---- main loop ----
            for b in range(B_LOC):
                exp_b = exp_pool.tile([1, T], F32)
                part_b = small_pool.tile([1, TS_PER_B], F32, tag="part")
                vlast = vlast_pool.tile([1, D], F32)
                nc.sync.dma_start(vlast[:], values.ap()[b:b + 1, T - 1, :])

                for tsub in range(TS_PER_B):
                    ts = b * TS_PER_B + tsub
                    tok0 = ts * TOK_TILE

                    if ts == 0:
                        vt = vt0
                    else:
                        vnat = vnat_pre[ts] if ts < NPREF else load_vnat(tok0)
                        vt = emit_transposes(vnat)

                    # h^T matmuls + tanh for all u-chunks, then the 8 score
                    # matmuls packed 4-way into distinct PE column groups
                    # (tile_position) so they run concurrently.  The bank is
                    # zeroed first and every packed matmul is accumulate-mode,
                    # which is correct whatever has_written state the bank
                    # carries; one e4 matmul then sums the 4 partial rows.
                    ps = psum_s.tile([128, TOK_TILE], F32, tag="ps_s")
                    nc.vector.memset(ps[:], 0.0)
                    ths = []
                    for u in range(UC):
                        ph = phs0[u] if (ts == 0 and u < 3) else emit_mains(vt, u)
                        if u == UC - 1:
                            # split the last tanh in two so the final score
                            # matmul starts half a tanh earlier (it was the
                            # one per-tile PE stall, ~300ns)
                            th = tanh_pool.tile([128, TOK_TILE], BF16,
                                                tag="th")
                            for hh in range(2):
                                nc.scalar.activation(
                                    th[:, hh * 256:(hh + 1) * 256],
                                    ph[:, hh * 256:(hh + 1) * 256], AFT.Tanh,
                                    bias=hbias[:, u * B_LOC + b:
                                               u * B_LOC + b + 1])
                            ths.append(th)
                        else:
                            ths.append(emit_tanh(ph, u, b))
                    for u in range(UC):
                        pos = 32 * (u % 4)
                        if u == UC - 1:
                            for hh in range(2):
                                nc.tensor.matmul(
                                    ps[pos:pos + 1, hh * 256:(hh + 1) * 256],
                                    vsb[:, u:u + 1],
                                    ths[u][:, hh * 256:(hh + 1) * 256],
                                    start=False, stop=False,
                                    tile_position=(0, pos),
                                    skip_group_check=True)
                        else:
                            nc.tensor.matmul(ps[pos:pos + 1, :],
                                             vsb[:, u:u + 1],
                                             ths[u][:], start=False,
                                             stop=False,
                                             tile_position=(0, pos),
                                             skip_group_check=True)
                    sc = scpy_pool.tile([128, TOK_TILE], BF16, tag="scpy")
                    nc.vector.tensor_copy(sc[:], ps[:])
                    nc.tensor.matmul(ps[0:1, :], e4[:], sc[:],
                                     start=True, stop=True,
                                     skip_group_check=True)

                    # exp(score) with fused per-tile sum
                    nc.scalar.activation(
                        exp_b[0:1, tsub * TOK_TILE:(tsub + 1) * TOK_TILE],
                        ps[0:1, :], AFT.Exp,
                        accum_out=part_b[0:1, tsub:tsub + 1])

                # batch epilogue: normalize, write attn + context
                bsum = small_pool.tile([1, 1], F32, tag="bsum")
                nc.vector.reduce_sum(bsum[:], part_b[:],
                                     axis=mybir.AxisListType.X)
                brecip = small_pool.tile([1, 1], F32, tag="brecip")
                nc.vector.reciprocal(brecip[:], bsum[:])

                # two chunks so the first half's store overlaps the second
                # half's multiply on the final batch's tail
                alast = small_pool.tile([1, 1], F32, tag="alast")
                nc.vector.tensor_scalar_mul(
                    alast[:], exp_b[0:1, T - 1:T], brecip[:])
                ctx = ctx_pool.tile([1, D], F32)
                nc.vector.tensor_scalar_mul(ctx[:], vlast[:], alast[:])
                nc.sync.dma_start(out_ctx.ap()[b:b + 1, :], ctx[:])

                attn = attn_pool.tile([1, T], F32)
                for hh in range(2):
                    sl = slice(hh * (T // 2), (hh + 1) * (T // 2))
                    nc.vector.tensor_scalar_mul(attn[0:1, sl],
                                                exp_b[0:1, sl], brecip[:])
                    nc.sync.dma_start(out_attn.ap()[b:b + 1, sl],
                                      attn[0:1, sl])

    nc.compile()
    return nc


_NC_CACHE = None


def _get_nc():
    global _NC_CACHE
    if _NC_CACHE is None:
        _NC_CACHE = build_nc()
    return _NC_CACHE


def kernel(**inputs) -> tuple[np.ndarray, np.ndarray]:
    nc = _get_nc()
    q = np.ascontiguousarray(np.asarray(inputs["query"], np.float32))
    vals = np.ascontiguousarray(np.asarray(inputs["values"], np.float32))
    shared = {
        "W1_k": np.ascontiguousarray(np.asarray(inputs["W1_k"], np.float32)),
        "W1_b": np.ascontiguousarray(np.asarray(inputs["W1_b"], np.float32)),
        "W2_k": np.ascontiguousarray(np.asarray(inputs["W2_k"], np.float32)),
        "W2_b": np.ascontiguousarray(np.asarray(inputs["W2_b"], np.float32)),
        "V_k": np.ascontiguousarray(np.asarray(inputs["V_k"], np.float32)),
    }
    in_maps = []
    for c in range(N_CORES):
        sl = slice(c * B_LOC, (c + 1) * B_LOC)
        in_maps.append({"values": vals[sl], "query": q[sl], **shared})

    res = run_bass_kernel_spmd(nc, in_maps, core_ids=list(range(N_CORES)))
    attn = np.concatenate([res.results[c]["out_attn"] for c in range(N_CORES)],
                          axis=0).reshape(B, T, 1)
    ctx = np.concatenate([res.results[c]["out_ctx"] for c in range(N_CORES)],
                         axis=0)
    return ctx.astype(np.float32), attn.astype(np.float32)


# revision 59
# speedup vs baseline: 1.0262x; 1.0262x over previous
"""Trainium2 Bass kernel for additive (Bahdanau-style) attention.

Reference computation (per batch b, over T time steps):
    h[b,t,:]  = values[b,t,:] @ W1_k + W1_b + (query[b,:] @ W2_k + W2_b)   # [U]
    score     = tanh(h) @ V_k + V_b                                        # [B,T,1]
    attn      = softmax(score, axis=1)
    context   = attn[:, T-1] * values[:, T-1]                              # [B,D]
    returns (context, attn)

Sharding: data-parallel over batch B=32 across 8 cores (B_loc=4/core),
weights replicated.  No collectives.

Kernel layout (per core): h is computed TRANSPOSED (h^T: U on partitions,
tokens on the free axis) so that
  - W1 is the matmul stationary operand in its natural [D, U] layout,
  - the U-reduction for the score is a PE matmul with lhsT = V[128,1],
  - softmax over T is a free-axis operation.
All inputs are loaded through HWDGE as f32 and cast to bf16 on the vector
engine (SWDGE descriptor generation on the Q7s serializes ~1us per DMA and
HWDGE transfers complete in issue order, so the emission order below is the
HBM arrival order).  values tiles are transposed on-chip via PE
transpose-mode; all matmuls run in bf16 with fp32 PSUM accumulation; the
softmax runs in fp32.  V_b is dropped: softmax is shift-invariant and score
itself is not an output.
"""

import sys
import types

import numpy as np

import concourse.bass as bass
import concourse.tile as tile
from concourse import bacc, mybir
from concourse import masks


def _ensure_ntff_hook_module():
    """bass_utils unconditionally imports antenv.axon_hooks when tracing is
    requested (e.g. BASS_TRACE=1); the image's antenv lacks it.  Register a
    compatible module so tracing works when the axon .so supports it and
    degrades to a logged no-op otherwise."""
    try:
        import antenv.axon_hooks  # noqa: F401
        return
    except ImportError:
        pass
    mod = types.ModuleType("antenv.axon_hooks")
    state = {"hook": None}
    mod.set_axon_ntff_profile_hook = lambda h: state.__setitem__("hook", h)
    mod.get_axon_ntff_profile_hook = lambda: state["hook"]
    sys.modules["antenv.axon_hooks"] = mod
    try:
        import antenv
        antenv.axon_hooks = mod
        from trn_agent_boot.trn_boot import _ntff_profile_via_ctypes
        mod.set_axon_ntff_profile_hook(
            _ntff_profile_via_ctypes("/opt/axon/libaxon_pjrt.so"))
    except Exception:
        pass  # hook stays None; bass_utils logs and skips tracing


_ensure_ntff_hook_module()
from concourse.bass_utils import run_bass_kernel_spmd  # noqa: E402

F32 = mybir.dt.float32
BF16 = mybir.dt.bfloat16
AFT = mybir.ActivationFunctionType

B, T, D, U = 32, 2048, 1024, 1024
N_CORES = 8
B_LOC = B // N_CORES          # 4 batches per core
TOKS = B_LOC * T              # 8192 tokens per core
TOK_TILE = 512                # tokens per compute tile
N_TS = TOKS // TOK_TILE       # 16 token tiles per core
TS_PER_B = T // TOK_TILE      # 4 token tiles per batch
KC = D // 128                 # 8 contraction chunks
UC = U // 128                 # 8 U chunks
NPREF = 3                     # token tiles prefetched before the main loop


def build_nc():
    nc = bacc.Bacc("TRN2", target_bir_lowering=False, debug=False,
                   num_devices=N_CORES)

    values = nc.dram_tensor("values", [B_LOC, T, D], F32, kind="ExternalInput")
    query = nc.dram_tensor("query", [B_LOC, D], F32, kind="ExternalInput")
    w1 = nc.dram_tensor("W1_k", [D, U], F32, kind="ExternalInput")
    w1b = nc.dram_tensor("W1_b", [U], F32, kind="ExternalInput")
    w2 = nc.dram_tensor("W2_k", [D, U], F32, kind="ExternalInput")
    w2b = nc.dram_tensor("W2_b", [U], F32, kind="ExternalInput")
    vk = nc.dram_tensor("V_k", [U, 1], F32, kind="ExternalInput")
    out_attn = nc.dram_tensor("out_attn", [B_LOC, T], F32, kind="ExternalOutput")
    out_ctx = nc.dram_tensor("out_ctx", [B_LOC, D], F32, kind="ExternalOutput")

    vals_flat = values.ap().rearrange("b t d -> (b t) d")

    from contextlib import ExitStack
    with tile.TileContext(nc) as tc:
        with ExitStack() as stk:
            ep = lambda **kw: stk.enter_context(tc.tile_pool(**kw))
            const_pool = ep(name="const", bufs=1)
            w1_pool = ep(name="w1p", bufs=1)
            w2_pool = ep(name="w2p", bufs=1)
            pre_pool = ep(name="pre", bufs=1)
            vstage_pool = ep(name="vstage", bufs=2)
            wstage_pool = ep(name="wstage", bufs=4)
            vnat_pool = ep(name="vnat", bufs=3)
            vt_pool = ep(name="vt", bufs=2 * KC)
            tanh_pool = ep(name="tanh", bufs=9)
            exp_pool = ep(name="expb", bufs=2)
            attn_pool = ep(name="attn", bufs=2)
            ctx_pool = ep(name="ctx", bufs=1)
            vlast_pool = ep(name="vlast", bufs=1)
            small_pool = ep(name="small", bufs=8)
            scpy_pool = ep(name="scpy", bufs=2)
            psum_t = ep(name="ps_t", bufs=3, space="PSUM")
            psum_h = ep(name="ps_h", bufs=3, space="PSUM")
            psum_s = ep(name="ps_s", bufs=2, space="PSUM")
            def cast_load(dst_slice, src_f32_ap, pool, tag, shape):
                stg = pool.tile(shape, F32, tag=tag)
                parts = src_f32_ap.shape[0]
                free = src_f32_ap.shape[-1]
                nc.sync.dma_start(stg[:parts, :free], src_f32_ap)
                nc.vector.tensor_copy(dst_slice, stg[:parts, :free])

            def load_vnat(tok0):
                """512-token values tile as ONE [128, 4*D] load (partition p
                holds tokens tok0+p, +128, +256, +384): single HWDGE DMA +
                single DVE bf16 cast amortize the ~0.6us per-DMA setup."""
                vn = vnat_pool.tile([128, 4 * D], BF16, tag="vnat")
                stg = vstage_pool.tile([128, 4 * D], F32, tag="vstage")
                nc.sync.dma_start(
                    stg[:].rearrange("p (j d) -> p j d", j=4),
                    vals_flat[tok0:tok0 + TOK_TILE, :].rearrange(
                        "(j p) d -> p j d", p=128))
                nc.vector.tensor_copy(vn[:], stg[:])
                return vn

            def emit_transposes(vn):
                """merged [128, 4*D] tile -> 8 [d-chunk, 512-token] tiles."""
                vt = []
                for k in range(KC):
                    pt = psum_t.tile([128, TOK_TILE], BF16, tag="ps_t")
                    for j in range(4):
                        nc.tensor.transpose(
                            pt[:, j * 128:(j + 1) * 128],
                            vn[:, j * D + k * 128: j * D + (k + 1) * 128],
                            ident[:])
                    v = vt_pool.tile([128, TOK_TILE], BF16, tag="vt")
                    nc.vector.tensor_copy(v[:], pt[:])
                    vt.append(v)
                return vt

            def emit_mains(vt, u):
                ph = psum_h.tile([128, TOK_TILE], F32, tag="ps_h")
                for k in range(KC):
                    nc.tensor.matmul(
                        ph[:],
                        w1sb[:, k * U + u * 128: k * U + (u + 1) * 128],
                        vt[k][:],
                        start=(k == 0), stop=(k == KC - 1))
                return ph

            def emit_tanh(ph, u, b):
                th = tanh_pool.tile([128, TOK_TILE], BF16, tag="th")
                nc.scalar.activation(
                    th[:], ph[:], AFT.Tanh,
                    bias=hbias[:, u * B_LOC + b: u * B_LOC + b + 1])
                return th

            # ---- preamble; emission order == HBM arrival order ----
            ident = const_pool.tile([128, 128], BF16)
            masks.make_identity(nc, ident[:])

            # e4[p] = 1 at p in {0,32,64,96}: reduces the 4 packed score rows
            e4a = const_pool.tile([128, 1], BF16)
            nc.vector.tensor_add(e4a[:], ident[:, 0:1], ident[:, 32:33])
            e4b = const_pool.tile([128, 1], BF16)
            nc.vector.tensor_add(e4b[:], ident[:, 64:65], ident[:, 96:97])
            e4 = const_pool.tile([128, 1], BF16)
            nc.vector.tensor_add(e4[:], e4a[:], e4b[:])

            qsb = const_pool.tile([B_LOC, D], BF16)
            cast_load(qsb[:], query.ap()[:, :], wstage_pool, "qstage",
                      [B_LOC, D])

            # biases as CONTIGUOUS [1,1024] rows (the "(u p) -> p u"
            # strided load costs 1024 4-byte descriptors); their add into h
            # rides the q2 matmul as a K=1 ones-row term.
            b1 = pre_pool.tile([1, U], F32)
            nc.sync.dma_start(b1[:], w1b.ap().rearrange("(o u) -> o u", o=1))
            b2 = pre_pool.tile([1, U], F32)
            nc.sync.dma_start(b2[:], w2b.ap().rearrange("(o u) -> o u", o=1))
            brow = pre_pool.tile([1, U], BF16)
            nc.vector.tensor_add(brow[:], b1[:], b2[:])
            ones1 = pre_pool.tile([1, B_LOC], BF16)
            nc.vector.memset(ones1[:], 1.0)

            vsb = const_pool.tile([128, UC], BF16)
            vkrow = pre_pool.tile([1, U], BF16)
            cast_load(vkrow[:], vk.ap().rearrange("u o -> o u"),
                      wstage_pool, "vkstage", [1, U])

            # values ts0, then W1 — everything the first main matmuls need
            vnat_pre = [load_vnat(0)]
            w1sb = w1_pool.tile([128, KC * U], BF16)
            for kg in range(2):
                stg = vstage_pool.tile([128, 4 * D], F32, tag="vstage")
                nc.sync.dma_start(
                    stg[:].rearrange("p (k u) -> p k u", k=4),
                    w1.ap()[kg * 512:(kg + 1) * 512, :].rearrange(
                        "(k p) u -> p k u", p=128))
                for k in range(4):
                    nc.vector.tensor_copy(
                        w1sb[:, (kg * 4 + k) * U: (kg * 4 + k + 1) * U],
                        stg[:, k * U:(k + 1) * U])

            # query^T (tiny PE work, can run as soon as qsb+ident land)
            qt = pre_pool.tile([128, KC * B_LOC], BF16)
            for k in range(KC):
                pq = psum_t.tile([128, B_LOC], BF16, tag="ps_t")
                nc.tensor.transpose(pq[:], qsb[:, k * 128:(k + 1) * 128],
                                    ident[0:B_LOC, 0:B_LOC])
                nc.vector.tensor_copy(qt[:, k * B_LOC:(k + 1) * B_LOC], pq[:])

            for u in range(UC):
                pv = psum_t.tile([128, 1], BF16, tag="ps_t")
                nc.tensor.transpose(pv[:], vkrow[0:1, u * 128:(u + 1) * 128],
                                    ident[0:1, 0:1])
                nc.vector.tensor_copy(vsb[:, u:u + 1], pv[:])

            # ts0: transposes + first three u-chunks of h^T, emitted BEFORE
            # the W2/q2 block so the PE starts as soon as W1+values land.
            hbias = const_pool.tile([128, UC * B_LOC], F32)
            vt0 = emit_transposes(vnat_pre[0])
            phs0 = [emit_mains(vt0, u) for u in range(3)]

            # W2 by column halves: hbias[u<4] only needs W2[:, 0:512], so the
            # first tanh unblocks after half the W2 traffic.
            # hbias[p, u*B_LOC+b] = q2[b, u*128+p] + W1_b[...] + W2_b[...]
            w2sb = w2_pool.tile([128, KC * U], BF16)
            q2sb = pre_pool.tile([B_LOC, U], BF16)
            for n in range(2):
                stg2 = vstage_pool.tile([128, 4 * D], F32, tag="vstage")
                nc.sync.dma_start(
                    stg2[:, 0:KC * 512].rearrange("p (k u) -> p k u", k=KC),
                    w2.ap()[:, n * 512:(n + 1) * 512].rearrange(
                        "(k p) u -> p k u", p=128))
                for k in range(KC):
                    nc.vector.tensor_copy(
                        w2sb[:, k * U + n * 512: k * U + (n + 1) * 512],
                        stg2[:, k * 512:(k + 1) * 512])
                pq2 = psum_s.tile([B_LOC, 512], F32, tag="ps_s")
                for k in range(KC):
                    nc.tensor.matmul(
                        pq2[:],
                        qt[:, k * B_LOC:(k + 1) * B_LOC],
                        w2sb[:, k * U + n * 512: k * U + (n + 1) * 512],
                        start=(k == 0), stop=False)
                nc.tensor.matmul(pq2[:], ones1[:],
                                 brow[0:1, n * 512:(n + 1) * 512],
                                 start=False, stop=True)
                nc.vector.tensor_copy(q2sb[:, n * 512:(n + 1) * 512], pq2[:])
                for u in range(n * 4, n * 4 + 4):
                    pq2t = psum_t.tile([128, B_LOC], BF16, tag="ps_t")
                    nc.tensor.transpose(pq2t[:], q2sb[:, u * 128:(u + 1) * 128],
                                        ident[0:B_LOC, 0:B_LOC])
                    nc.scalar.activation(
                        hbias[:, u * B_LOC:(u + 1) * B_LOC], pq2t[:],
                        AFT.Identity)
                if n == 0:
                    vnat_pre.append(load_vnat(TOK_TILE))

            for ts in range(2, NPREF):
                vnat_pre.append(load_vnat(ts * TOK_TILE))

            # ---- main loop (score tail software-pipelined) ----
            # The final score matmul of tile ts waits on tanh(u=7); emitting
            # the NEXT tile's transposes first fills that ~300ns PE stall.
            def finish_scores(pend):
                th7, ps_p, exp_p, part_p, tsub_p = pend
                nc.tensor.matmul(ps_p[96:97, :], vsb[:, UC - 1:UC], th7[:],
                                 start=False, stop=False,
                                 tile_position=(0, 96),
                                 skip_group_check=True)
                sc = scpy_pool.tile([128, TOK_TILE], BF16, tag="scpy")
                nc.vector.tensor_copy(sc[:], ps_p[:])
                nc.tensor.matmul(ps_p[0:1, :], e4[:], sc[:],
                                 start=True, stop=True,
                                 skip_group_check=True)
                nc.scalar.activation(
                    exp_p[0:1, tsub_p * TOK_TILE:(tsub_p + 1) * TOK_TILE],
                    ps_p[0:1, :], AFT.Exp,
                    accum_out=part_p[0:1, tsub_p:tsub_p + 1])

            def epilogue(b, exp_p, part_p, vlast_p):
                bsum = small_pool.tile([1, 1], F32, tag="bsum")
                nc.vector.reduce_sum(bsum[:], part_p[:],
                                     axis=mybir.AxisListType.X)
                brecip = small_pool.tile([1, 1], F32, tag="brecip")
                nc.vector.reciprocal(brecip[:], bsum[:])

                alast = small_pool.tile([1, 1], F32, tag="alast")
                nc.vector.tensor_scalar_mul(
                    alast[:], exp_p[0:1, T - 1:T], brecip[:])
                ctx = ctx_pool.tile([1, D], F32)
                nc.vector.tensor_scalar_mul(ctx[:], vlast_p[:], alast[:])
                nc.sync.dma_start(out_ctx.ap()[b:b + 1, :], ctx[:])

                attn = attn_pool.tile([1, T], F32)
                for hh in range(2):
                    sl = slice(hh * (T // 2), (hh + 1) * (T // 2))
                    nc.vector.tensor_scalar_mul(attn[0:1, sl],
                                                exp_p[0:1, sl], brecip[:])
                    nc.sync.dma_start(out_attn.ap()[b:b + 1, sl],
                                      attn[0:1, sl])

            pending = None
            batch_state = None  # (b, exp_b, part_b, vlast)
            for ts in range(N_TS):
                b, tsub = divmod(ts, TS_PER_B)

                if ts == 0:
                    vt = vt0
                else:
                    vnat = vnat_pre[ts] if ts < NPREF else load_vnat(ts * TOK_TILE)
                    vt = emit_transposes(vnat)

                if pending is not None:
                    finish_scores(pending)
                    pending = None
                    if tsub == 0:
                        epilogue(*[batch_state[0], batch_state[1],
                                   batch_state[2], batch_state[3]])

                if tsub == 0:
                    exp_b = exp_pool.tile([1, T], F32)
                    part_b = small_pool.tile([1, TS_PER_B], F32, tag="part")
                    vlast = vlast_pool.tile([1, D], F32)
                    nc.sync.dma_start(vlast[:],
                                      values.ap()[b:b + 1, T - 1, :])
                    batch_state = (b, exp_b, part_b, vlast)

                ps = psum_s.tile([128, TOK_TILE], F32, tag="ps_s")
                nc.vector.memset(ps[:], 0.0)
                ths = []
                for u in range(UC):
                    ph = phs0[u] if (ts == 0 and u < 3) else emit_mains(vt, u)
                    ths.append(emit_tanh(ph, u, b))
                for u in range(UC - 1):
                    pos = 32 * (u % 4)
                    nc.tensor.matmul(ps[pos:pos + 1, :], vsb[:, u:u + 1],
                                     ths[u][:], start=False, stop=False,
                                     tile_position=(0, pos),
                                     skip_group_check=True)
                pending = (ths[UC - 1], ps, exp_b, part_b, tsub)

            finish_scores(pending)
            epilogue(batch_state[0], batch_state[1], batch_state[2],
                     batch_state[3])

    nc.compile()
    return nc


_NC_CACHE = None


def _get_nc():
    global _NC_CACHE
    if _NC_CACHE is None:
        _NC_CACHE = build_nc()
    return _NC_CACHE


def kernel(**inputs) -> tuple[np.ndarray, np.ndarray]:
    nc = _get_nc()
    q = np.ascontiguousarray(np.asarray(inputs["query"], np.float32))
    vals = np.ascontiguousarray(np.asarray(inputs["values"], np.float32))
    shared = {
        "W1_k": np.ascontiguousarray(np.asarray(inputs["W1_k"], np.float32)),
        "W1_b": np.ascontiguousarray(np.asarray(inputs["W1_b"], np.float32)),
        "W2_k": np.ascontiguousarray(np.asarray(inputs["W2_k"], np.float32)),
        "W2_b": np.ascontiguousarray(np.asarray(inputs["W2_b"], np.float32)),
        "V_k": np.ascontiguousarray(np.asarray(inputs["V_k"], np.float32)),
    }
    in_maps = []
    for c in range(N_CORES):
        sl = slice(c * B_LOC, (c + 1) * B_LOC)
        in_maps.append({"values": vals[sl], "query": q[sl], **shared})

    res = run_bass_kernel_spmd(nc, in_maps, core_ids=list(range(N_CORES)))
    attn = np.concatenate([res.results[c]["out_attn"] for c in range(N_CORES)],
                          axis=0).reshape(B, T, 1)
    ctx = np.concatenate([res.results[c]["out_ctx"] for c in range(N_CORES)],
                         axis=0)
    return ctx.astype(np.float32), attn.astype(np.float32)
